# revision 12
# baseline (speedup 1.0000x reference)
"""Single-head attention on 8 Trainium2 NeuronCores, batch-sharded.

Per core (one batch element b), with x fed pre-transposed as xT [768, 2048]:

  q^T/v^T    via one fused [Wq|Wv] matmul (M=128, full PE array), k^T via
             its own M=64 matmul. All projections fp32r, K=128, N=512.
  q^T, k^T   stored bf16 in rows 0-63 of zero-padded [128, 2048] tiles:
             K=128 contractions stream 2x faster than K=64 on TRN2, and
             the zero rows contribute nothing.
  v^T        f32 in rows 64-127 (where the fused matmul puts it), then
             PE-transposed per k-tile into V [k, h] bf16 with an extra
             ones column.
  scores^T   [k-tile=128, q] = K^T-tile x Q^T      (PE bf16 K=128 N=512)
  P^T        = exp(scores^T / 8)                   (ACT -> bf16, 1024-wide;
             no max-subtraction: |scores/8| <~ 2, exp is safe)
  out^T,den  = [V | 1] x P^T accumulated over k    (PE bf16; the ones
             column yields the softmax denominator row)
  out        = transpose(out^T) rows / denominator (PE + DVE), one DMA
             per 512-row block.

Scheduling: xT arrives in s-column chunks; the projection phase is
chunk-interleaved and also carries the ENTIRE first q-chunk of the
attention loop (scores+exp+PV), so PE and ACT are both busy while DMA
streams. Remaining q-chunks run back-to-back, each epilogue hiding in
the next chunk's window. PE never idles long enough for the HAM clock
gate to re-throttle. No inter-core communication.
"""

import numpy as np

B, S, D, H = 8, 2048, 768, 64
DT = D // 128  # 6 d-tiles
NQ = S // 512  # 4 q-chunks of 512
NK = S // 128  # 16 k-tiles of 128
NKP = NK // 2  # 8 k-tile pairs (one 1024-wide exp each)
SCALE = 1.0 / np.sqrt(H).item()

_cache = {}


def _build():
    import concourse.mybir as mybir
    import concourse.tile as tile
    from concourse import bacc
    from concourse.masks import make_identity

    f32 = mybir.dt.float32
    f32r = mybir.dt.float32r
    bf16 = mybir.dt.bfloat16
    Exp = mybir.ActivationFunctionType.Exp

    nc = bacc.Bacc(None)
    xT_d = nc.dram_tensor("xT", [D, S], f32, kind="ExternalInput")
    wq_d = nc.dram_tensor("wq", [D, H], f32, kind="ExternalInput")
    wk_d = nc.dram_tensor("wk", [D, H], f32, kind="ExternalInput")
    wv_d = nc.dram_tensor("wv", [D, H], f32, kind="ExternalInput")
    bq_d = nc.dram_tensor("bq", [H, 1], f32, kind="ExternalInput")
    bk_d = nc.dram_tensor("bk", [H, 1], f32, kind="ExternalInput")
    bv_d = nc.dram_tensor("bv", [H, 1], f32, kind="ExternalInput")
    ones_d = nc.dram_tensor("ones", [128, NK], bf16, kind="ExternalInput")
    out_d = nc.dram_tensor("out", [S, H], f32, kind="ExternalOutput")

    with tile.TileContext(nc) as tc:
        with (
            tc.tile_pool(name="big", bufs=1) as big,
            tc.tile_pool(name="small", bufs=1) as small,
            tc.tile_pool(name="pt", bufs=3) as ptp,
            tc.tile_pool(name="res", bufs=2) as resp,
            tc.tile_pool(name="psA", bufs=3, space="PSUM") as psA,
            tc.tile_pool(name="psO", bufs=2, space="PSUM") as psO,
        ):
            # ---- constants / weights ----
            ident = small.tile([128, 128], f32)
            make_identity(nc, ident)
            identb = small.tile([128, 128], bf16)
            nc.gpsimd.tensor_copy(out=identb, in_=ident)

            # fused [Wq | Wv] -> psum rows 0-63 = q, rows 64-127 = v
            wqv = small.tile([128, DT, 128], f32r)
            nc.sync.dma_start(
                out=wqv[:, :, :H],
                in_=wq_d[:, :].rearrange("(t p) h -> p t h", p=128).bitcast(f32r),
            )
            nc.sync.dma_start(
                out=wqv[:, :, H:],
                in_=wv_d[:, :].rearrange("(t p) h -> p t h", p=128).bitcast(f32r),
            )
            wk = small.tile([128, DT, H], f32r)
            nc.sync.dma_start(
                out=wk,
                in_=wk_d[:, :].rearrange("(t p) h -> p t h", p=128).bitcast(f32r),
            )
            # bias vector aligned with psum rows: [bq; bv] and bk
            bqv = small.tile([128, 1], f32)
            nc.sync.dma_start(out=bqv[:H, :], in_=bq_d[:, :])
            nc.sync.dma_start(out=bqv[H:, :], in_=bv_d[:, :])
            bk = small.tile([H, 1], f32)
            nc.sync.dma_start(out=bk, in_=bk_d[:, :])

            # ---- x^T, DMA'd in s-column chunks so early chunks arrive fast
            xT = big.tile([128, DT, S], f32r)
            for c in range(NQ):
                for dt in range(DT):
                    nc.sync.dma_start(
                        out=xT[:, dt, c * 512 : (c + 1) * 512],
                        in_=xT_d[
                            dt * 128 : (dt + 1) * 128, c * 512 : (c + 1) * 512
                        ].bitcast(f32r),
                    )

            # q/k: [128, 2048] bf16, data rows 0-63, rows 64-127 zeroed.
            qT = big.tile([128, S], bf16, tag="qT")
            kT = big.tile([128, S], bf16, tag="kT")
            vT = big.tile([128, S], f32, tag="vT")  # data in rows 64-127
            vTlo = big.tile([H, S], f32, tag="vTlo")  # DMA-realigned to rows 0-63
            nc.gpsimd.memset(qT[H:128, :], 0.0)
            nc.gpsimd.memset(kT[H:128, :], 0.0)

            v65 = big.tile([128, NK, H + 1], bf16)
            nc.sync.dma_start(out=v65[:, :, H : H + 1], in_=ones_d[:, :].unsqueeze(2))

            outqs = [None] * NQ
            pT0 = [None] * NKP  # first q-chunk P^T pair-tiles

            def emit_s_exp(kp, qc, pstore):
                """scores for k-tile pair kp against q-chunk qc + 1024-wide exp."""
                sc = psA.tile([128, 1024], f32, tag="a", name=f"sc{qc}_{kp}")
                for h2 in range(2):
                    kt = kp * 2 + h2
                    nc.tensor.matmul(
                        sc[:, h2 * 512 : (h2 + 1) * 512],
                        lhsT=kT[:, kt * 128 : (kt + 1) * 128],
                        rhs=qT[:, qc * 512 : (qc + 1) * 512],
                        start=True,
                        stop=True,
                    )
                pt = ptp.tile([128, 1024], bf16, tag="pT", name=f"pt{qc}_{kp}")
                nc.scalar.activation(out=pt, in_=sc, func=Exp, scale=SCALE)
                pstore[kp] = pt

            def emit_pv(kp, qc):
                for h2 in range(2):
                    kt = kp * 2 + h2
                    nc.tensor.matmul(
                        outqs[qc],
                        lhsT=v65[:, kt, :],
                        rhs=(pT0 if qc == 0 else pTq)[kp][
                            :, h2 * 512 : (h2 + 1) * 512
                        ],
                        start=(kt == 0),
                        stop=(kt == NK - 1),
                    )

            def emit_epilogue(qc):
                oTq = resp.tile([H + 1, 512], bf16, tag="oT", name=f"oT{qc}")
                nc.vector.tensor_copy(out=oTq, in_=outqs[qc])
                tp4 = psA.tile([128, 4, H + 2], bf16, tag="a", name=f"tp4_{qc}")
                for st in range(4):
                    nc.tensor.transpose(
                        tp4[:, st, : H + 1],
                        oTq[:, st * 128 : (st + 1) * 128],
                        identb[: H + 1, : H + 1],
                    )
                rec = resp.tile([128, 4, 1], f32, tag="rec", name=f"rec{qc}")
                nc.vector.reciprocal(out=rec, in_=tp4[:, :, H : H + 1])
                res = resp.tile([128, 4, H], f32, tag="res", name=f"res{qc}")
                nc.vector.tensor_mul(
                    out=res, in0=tp4[:, :, :H], in1=rec.broadcast_to([128, 4, H])
                )
                nc.sync.dma_start(
                    out=out_d[qc * 512 : (qc + 1) * 512, :].rearrange(
                        "(st p) h -> p st h", p=128
                    ),
                    in_=res,
                )

            # ---- projection phase, chunk-interleaved, carrying qc=0 work
            outqs[0] = psO.tile([H + 1, 512], f32, tag="o", name="outq0")
            for c in range(NQ):
                # fused q+v projection for this s-chunk (M=128)
                psqv = psA.tile([128, 512], f32, tag="a", name=f"psqv{c}")
                for dt in range(DT):
                    nc.tensor.matmul(
                        psqv,
                        lhsT=wqv[:, dt, :],
                        rhs=xT[:, dt, c * 512 : (c + 1) * 512],
                        start=(dt == 0),
                        stop=(dt == DT - 1),
                    )
                nc.vector.tensor_scalar_add(
                    out=qT[:H, c * 512 : (c + 1) * 512],
                    in0=psqv[:H, :],
                    scalar1=bqv[:H, :],
                )
                nc.vector.tensor_scalar_add(
                    out=vT[H:, c * 512 : (c + 1) * 512],
                    in0=psqv[H:, :],
                    scalar1=bqv[H:, :],
                )
                # k projection (M=64)
                psk = psA.tile([H, 512], f32, tag="a", name=f"psk{c}")
                for dt in range(DT):
                    nc.tensor.matmul(
                        psk,
                        lhsT=wk[:, dt, :],
                        rhs=xT[:, dt, c * 512 : (c + 1) * 512],
                        start=(dt == 0),
                        stop=(dt == DT - 1),
                    )
                nc.vector.tensor_scalar_add(
                    out=kT[:H, c * 512 : (c + 1) * 512], in0=psk, scalar1=bk
                )
                # realign v rows 64-127 -> 0-63 (SBUF->SBUF DMA), then
                # V transposes for this chunk's 4 k-tiles at base partition 0
                nc.sync.dma_start(
                    out=vTlo[:, c * 512 : (c + 1) * 512],
                    in_=vT[H:, c * 512 : (c + 1) * 512],
                )
                for j in range(4):
                    kt = c * 4 + j
                    tp = psA.tile([128, H], f32, tag="a", name=f"vtr{kt}")
                    nc.tensor.transpose(
                        tp,
                        vTlo[:, kt * 128 : (kt + 1) * 128],
                        ident[:H, :H],
                    )
                    nc.vector.tensor_copy(out=v65[:, kt, :H], in_=tp)
                # first q-chunk attention for this chunk's k-tiles
                for kp in (2 * c, 2 * c + 1):
                    emit_s_exp(kp, 0, pT0)
                    emit_pv(kp, 0)

            emit_epilogue(0)

            # ---- remaining q-chunks, PV pipelined one pair behind ----
            for qc in range(1, NQ):
                pTq = [None] * NKP
                outqs[qc] = psO.tile(
                    [H + 1, 512], f32, tag="o", name=f"outq{qc}"
                )
                for kp in range(NKP + 1):
                    if kp < NKP:
                        emit_s_exp(kp, qc, pTq)
                    if kp >= 1:
                        emit_pv(kp - 1, qc)
                emit_epilogue(qc)

    nc.compile()
    return nc


def _get_nc():
    if "nc" not in _cache:
        _cache["nc"] = _build()
    return _cache["nc"]


def _ones_bf16():
    import ml_dtypes

    return np.ones((128, NK), ml_dtypes.bfloat16)


def kernel(x, Wq, bq, Wk, bk, Wv, bv, **_):
    from concourse.bass_utils import run_bass_kernel_spmd

    nc = _get_nc()
    x = np.asarray(x, dtype=np.float32)
    common = {
        "wq": np.ascontiguousarray(np.asarray(Wq, np.float32)),
        "wk": np.ascontiguousarray(np.asarray(Wk, np.float32)),
        "wv": np.ascontiguousarray(np.asarray(Wv, np.float32)),
        "bq": np.ascontiguousarray(np.asarray(bq, np.float32).reshape(H, 1)),
        "bk": np.ascontiguousarray(np.asarray(bk, np.float32).reshape(H, 1)),
        "bv": np.ascontiguousarray(np.asarray(bv, np.float32).reshape(H, 1)),
        "ones": _ones_bf16(),
    }
    in_maps = [
        {"xT": np.ascontiguousarray(x[b].T), **common} for b in range(B)
    ]
    res = run_bass_kernel_spmd(nc, in_maps, core_ids=list(range(B)))
    return np.stack([res.results[b]["out"] for b in range(B)])


# revision 14
# speedup vs baseline: 1.0346x; 1.0346x over previous
"""Single-head attention on 8 Trainium2 NeuronCores, batch-sharded.

Per core (one batch element b), with x fed pre-transposed as xT [768, 2048]:

  q^T/v^T    via one fused [Wq|Wv] matmul (M=128, full PE array), k^T via
             its own M=64 matmul. All projections fp32r, K=128, N=512.
  q^T, k^T   stored bf16 in rows 0-63 of zero-padded [128, 2048] tiles:
             K=128 contractions stream 2x faster than K=64 on TRN2, and
             the zero rows contribute nothing.
  v^T        f32 in rows 64-127 (where the fused matmul puts it), then
             PE-transposed per k-tile into V [k, h] bf16 with an extra
             ones column.
  scores^T   [k-tile=128, q] = K^T-tile x Q^T      (PE bf16 K=128 N=512)
  P^T        = exp(scores^T / 8)                   (ACT -> bf16, 1024-wide;
             no max-subtraction: |scores/8| <~ 2, exp is safe)
  out^T,den  = [V | 1] x P^T accumulated over k    (PE bf16; the ones
             column yields the softmax denominator row)
  out        = transpose(out^T) rows / denominator (PE + DVE), one DMA
             per 512-row block.

Scheduling: xT arrives in s-column chunks; the projection phase is
chunk-interleaved and also carries the ENTIRE first q-chunk of the
attention loop (scores+exp+PV), so PE and ACT are both busy while DMA
streams. Remaining q-chunks run back-to-back, each epilogue hiding in
the next chunk's window. PE never idles long enough for the HAM clock
gate to re-throttle. No inter-core communication.
"""

import numpy as np

B, S, D, H = 8, 2048, 768, 64
DT = D // 128  # 6 d-tiles
NQ = S // 512  # 4 q-chunks of 512
NK = S // 128  # 16 k-tiles of 128
NKP = NK // 2  # 8 k-tile pairs (one 1024-wide exp each)
SCALE = 1.0 / np.sqrt(H).item()

_cache = {}


def _build():
    import concourse.mybir as mybir
    import concourse.tile as tile
    from concourse import bacc
    from concourse.masks import make_identity

    f32 = mybir.dt.float32
    f32r = mybir.dt.float32r
    bf16 = mybir.dt.bfloat16
    Exp = mybir.ActivationFunctionType.Exp

    nc = bacc.Bacc(None)
    xT_d = nc.dram_tensor("xT", [D, S], f32, kind="ExternalInput")
    wq_d = nc.dram_tensor("wq", [D, H], f32, kind="ExternalInput")
    wk_d = nc.dram_tensor("wk", [D, H], f32, kind="ExternalInput")
    wv_d = nc.dram_tensor("wv", [D, H], f32, kind="ExternalInput")
    bq_d = nc.dram_tensor("bq", [H, 1], f32, kind="ExternalInput")
    bk_d = nc.dram_tensor("bk", [H, 1], f32, kind="ExternalInput")
    bv_d = nc.dram_tensor("bv", [H, 1], f32, kind="ExternalInput")
    ones_d = nc.dram_tensor("ones", [128, NK], bf16, kind="ExternalInput")
    out_d = nc.dram_tensor("out", [S, H], f32, kind="ExternalOutput")

    with tile.TileContext(nc) as tc:
        with (
            tc.tile_pool(name="big", bufs=1) as big,
            tc.tile_pool(name="small", bufs=1) as small,
            tc.tile_pool(name="pt", bufs=3) as ptp,
            tc.tile_pool(name="res", bufs=2) as resp,
            tc.tile_pool(name="psA", bufs=3, space="PSUM") as psA,
            tc.tile_pool(name="psO", bufs=2, space="PSUM") as psO,
        ):
            # ---- constants / weights ----
            ident = small.tile([128, 128], f32)
            make_identity(nc, ident)
            identb = small.tile([128, 128], bf16)
            nc.gpsimd.tensor_copy(out=identb, in_=ident)

            # fused [Wq | Wv] -> psum rows 0-63 = q, rows 64-127 = v
            wqv = small.tile([128, DT, 128], f32r)
            nc.sync.dma_start(
                out=wqv[:, :, :H],
                in_=wq_d[:, :].rearrange("(t p) h -> p t h", p=128).bitcast(f32r),
            )
            nc.sync.dma_start(
                out=wqv[:, :, H:],
                in_=wv_d[:, :].rearrange("(t p) h -> p t h", p=128).bitcast(f32r),
            )
            wk = small.tile([128, DT, H], f32r)
            nc.sync.dma_start(
                out=wk,
                in_=wk_d[:, :].rearrange("(t p) h -> p t h", p=128).bitcast(f32r),
            )
            # bias vector aligned with psum rows: [bq; bv] and bk
            bqv = small.tile([128, 1], f32)
            nc.sync.dma_start(out=bqv[:H, :], in_=bq_d[:, :])
            nc.sync.dma_start(out=bqv[H:, :], in_=bv_d[:, :])
            bk = small.tile([H, 1], f32)
            nc.sync.dma_start(out=bk, in_=bk_d[:, :])

            # ---- x^T, DMA'd in s-column chunks so early chunks arrive fast
            xT = big.tile([128, DT, S], f32r)
            for c in range(NQ):
                for dt in range(DT):
                    nc.sync.dma_start(
                        out=xT[:, dt, c * 512 : (c + 1) * 512],
                        in_=xT_d[
                            dt * 128 : (dt + 1) * 128, c * 512 : (c + 1) * 512
                        ].bitcast(f32r),
                    )

            # q/k: [128, 2048] bf16, data rows 0-63, rows 64-127 zeroed.
            qT = big.tile([128, S], bf16, tag="qT")
            kT = big.tile([128, S], bf16, tag="kT")
            vT = big.tile([128, S], f32, tag="vT")  # data in rows 64-127
            vTlo = big.tile([H, S], f32, tag="vTlo")  # DMA-realigned to rows 0-63
            nc.gpsimd.memset(qT[H:128, :], 0.0)
            nc.gpsimd.memset(kT[H:128, :], 0.0)

            v65 = big.tile([128, NK, H + 1], bf16)
            nc.sync.dma_start(out=v65[:, :, H : H + 1], in_=ones_d[:, :].unsqueeze(2))

            outqs = [None] * NQ
            pT0 = [None] * NKP  # first q-chunk P^T pair-tiles

            def emit_s_exp(kp, qc, pstore):
                """scores for k-tile pair kp against q-chunk qc + 1024-wide exp."""
                sc = psA.tile([128, 1024], f32, tag="a", name=f"sc{qc}_{kp}")
                for h2 in range(2):
                    kt = kp * 2 + h2
                    nc.tensor.matmul(
                        sc[:, h2 * 512 : (h2 + 1) * 512],
                        lhsT=kT[:, kt * 128 : (kt + 1) * 128],
                        rhs=qT[:, qc * 512 : (qc + 1) * 512],
                        start=True,
                        stop=True,
                    )
                pt = ptp.tile([128, 1024], bf16, tag="pT", name=f"pt{qc}_{kp}")
                nc.scalar.activation(out=pt, in_=sc, func=Exp, scale=SCALE)
                pstore[kp] = pt

            def emit_pv(kp, qc):
                for h2 in range(2):
                    kt = kp * 2 + h2
                    nc.tensor.matmul(
                        outqs[qc],
                        lhsT=v65[:, kt, :],
                        rhs=(pT0 if qc == 0 else pTq)[kp][
                            :, h2 * 512 : (h2 + 1) * 512
                        ],
                        start=(kt == 0),
                        stop=(kt == NK - 1),
                    )

            def emit_epilogue(qc):
                oTq = resp.tile([H + 1, 512], bf16, tag="oT", name=f"oT{qc}")
                nc.vector.tensor_copy(out=oTq, in_=outqs[qc])
                tp4 = psA.tile([128, 4, H + 2], bf16, tag="a", name=f"tp4_{qc}")
                for st in range(4):
                    nc.tensor.transpose(
                        tp4[:, st, : H + 1],
                        oTq[:, st * 128 : (st + 1) * 128],
                        identb[: H + 1, : H + 1],
                    )
                rec = resp.tile([128, 4, 1], f32, tag="rec", name=f"rec{qc}")
                nc.vector.reciprocal(out=rec, in_=tp4[:, :, H : H + 1])
                res = resp.tile([128, 4, H], f32, tag="res", name=f"res{qc}")
                nc.vector.tensor_mul(
                    out=res, in0=tp4[:, :, :H], in1=rec.broadcast_to([128, 4, H])
                )
                nc.sync.dma_start(
                    out=out_d[qc * 512 : (qc + 1) * 512, :].rearrange(
                        "(st p) h -> p st h", p=128
                    ),
                    in_=res,
                )

            # ---- projection phase, chunk-interleaved, carrying qc=0 work
            outqs[0] = psO.tile([H + 1, 512], f32, tag="o", name="outq0")
            for c in range(NQ):
                # fused q+v projection for this s-chunk (M=128)
                psqv = psA.tile([128, 512], f32, tag="a", name=f"psqv{c}")
                for dt in range(DT):
                    nc.tensor.matmul(
                        psqv,
                        lhsT=wqv[:, dt, :],
                        rhs=xT[:, dt, c * 512 : (c + 1) * 512],
                        start=(dt == 0),
                        stop=(dt == DT - 1),
                    )
                nc.vector.tensor_scalar_add(
                    out=qT[:H, c * 512 : (c + 1) * 512],
                    in0=psqv[:H, :],
                    scalar1=bqv[:H, :],
                )
                nc.vector.tensor_scalar_add(
                    out=vT[H:, c * 512 : (c + 1) * 512],
                    in0=psqv[H:, :],
                    scalar1=bqv[H:, :],
                )
                # k projection (M=64)
                psk = psA.tile([H, 512], f32, tag="a", name=f"psk{c}")
                for dt in range(DT):
                    nc.tensor.matmul(
                        psk,
                        lhsT=wk[:, dt, :],
                        rhs=xT[:, dt, c * 512 : (c + 1) * 512],
                        start=(dt == 0),
                        stop=(dt == DT - 1),
                    )
                nc.vector.tensor_scalar_add(
                    out=kT[:H, c * 512 : (c + 1) * 512], in0=psk, scalar1=bk
                )
                # realign v rows 64-127 -> 0-63 (SBUF->SBUF DMA), then
                # V transposes for this chunk's 4 k-tiles at base partition 0
                nc.sync.dma_start(
                    out=vTlo[:, c * 512 : (c + 1) * 512],
                    in_=vT[H:, c * 512 : (c + 1) * 512],
                )
                for j in range(4):
                    kt = c * 4 + j
                    tp = psA.tile([128, H], f32, tag="a", name=f"vtr{kt}")
                    nc.tensor.transpose(
                        tp,
                        vTlo[:, kt * 128 : (kt + 1) * 128],
                        ident[:H, :H],
                    )
                    nc.vector.tensor_copy(out=v65[:, kt, :H], in_=tp)
                # first q-chunk attention for this chunk's k-tiles;
                # PVs run one chunk behind so exp has a full window of lead
                for kp in (2 * c, 2 * c + 1):
                    emit_s_exp(kp, 0, pT0)
                if c >= 1:
                    emit_pv(2 * c - 2, 0)
                    emit_pv(2 * c - 1, 0)
            emit_pv(NKP - 2, 0)
            emit_pv(NKP - 1, 0)

            emit_epilogue(0)

            # ---- remaining q-chunks, PV pipelined one pair behind ----
            for qc in range(1, NQ):
                pTq = [None] * NKP
                outqs[qc] = psO.tile(
                    [H + 1, 512], f32, tag="o", name=f"outq{qc}"
                )
                for kp in range(NKP + 2):
                    if kp < NKP:
                        emit_s_exp(kp, qc, pTq)
                    if kp >= 2:
                        emit_pv(kp - 2, qc)
                emit_epilogue(qc)

    nc.compile()
    return nc


def _get_nc():
    if "nc" not in _cache:
        _cache["nc"] = _build()
    return _cache["nc"]


def _ones_bf16():
    import ml_dtypes

    return np.ones((128, NK), ml_dtypes.bfloat16)


def kernel(x, Wq, bq, Wk, bk, Wv, bv, **_):
    from concourse.bass_utils import run_bass_kernel_spmd

    nc = _get_nc()
    x = np.asarray(x, dtype=np.float32)
    common = {
        "wq": np.ascontiguousarray(np.asarray(Wq, np.float32)),
        "wk": np.ascontiguousarray(np.asarray(Wk, np.float32)),
        "wv": np.ascontiguousarray(np.asarray(Wv, np.float32)),
        "bq": np.ascontiguousarray(np.asarray(bq, np.float32).reshape(H, 1)),
        "bk": np.ascontiguousarray(np.asarray(bk, np.float32).reshape(H, 1)),
        "bv": np.ascontiguousarray(np.asarray(bv, np.float32).reshape(H, 1)),
        "ones": _ones_bf16(),
    }
    in_maps = [
        {"xT": np.ascontiguousarray(x[b].T), **common} for b in range(B)
    ]
    res = run_bass_kernel_spmd(nc, in_maps, core_ids=list(range(B)))
    return np.stack([res.results[b]["out"] for b in range(B)])


# revision 16
# speedup vs baseline: 1.2477x; 1.2060x over previous
"""Single-head attention on 8 Trainium2 NeuronCores, batch-sharded.

Per core (one batch element b), with x fed pre-transposed as xT [768, 2048]:

  v^T/q^T    via one fused [Wv|Wq] matmul (M=128, full PE array): psum
             rows 0-63 = v^T, rows 64-127 = q^T.
  k^T        via a host-padded [0|Wk] matmul (M=128): psum rows 0-63 = 0,
             rows 64-127 = k^T.
  q^T, k^T   stored bf16 in rows 64-127 of [128, 2048] tiles with zeros
             in rows 0-63: the K=128 contraction streams 2x faster than
             K=64 on TRN2, zeros pair with zeros, and every engine copy
             stays partition-aligned (no cross-partition moves).
  v^T        f32 rows 0-63, PE-transposed per k-tile into V [k, h] bf16
             with an appended ones column.
  scores^T   [k-tile=128, q] = K^T-tile x Q^T      (PE bf16 K=128 N=512)
  P^T        = exp(scores^T / 8)                   (ACT -> bf16, 1024-wide;
             no max-subtraction: |scores/8| <~ 2, exp is safe)
  out^T,den  = [V | 1] x P^T accumulated over k    (PE bf16; the ones
             column yields the softmax denominator row)
  out        = transpose(out^T) rows / denominator (PE + DVE), one DMA
             per 512-row block.

Scheduling: xT arrives in s-column chunks; the projection phase is
chunk-interleaved and also carries the first q-chunk's scores/exp (PVs
one chunk behind), so PE and ACT are busy while DMA streams. Remaining
q-chunks run with PV two k-pairs behind the scores, epilogues hiding in
the next chunk's window. PE never idles long enough for the HAM clock
gate to re-throttle. No inter-core communication.
"""

import numpy as np

B, S, D, H = 8, 2048, 768, 64
DT = D // 128  # 6 d-tiles
NQ = S // 512  # 4 q-chunks of 512
NK = S // 128  # 16 k-tiles of 128
NKP = NK // 2  # 8 k-tile pairs (one 1024-wide exp each)
SCALE = 1.0 / np.sqrt(H).item()

_cache = {}


def _build():
    import concourse.mybir as mybir
    import concourse.tile as tile
    from concourse import bacc
    from concourse.masks import make_identity

    f32 = mybir.dt.float32
    f32r = mybir.dt.float32r
    bf16 = mybir.dt.bfloat16
    Exp = mybir.ActivationFunctionType.Exp

    nc = bacc.Bacc(None)
    xT_d = nc.dram_tensor("xT", [D, S], f32, kind="ExternalInput")
    wvq_d = nc.dram_tensor("wvq", [D, 128], f32, kind="ExternalInput")
    wk0_d = nc.dram_tensor("wk0", [D, 128], f32, kind="ExternalInput")
    bvq_d = nc.dram_tensor("bvq", [128, 1], f32, kind="ExternalInput")
    bk0_d = nc.dram_tensor("bk0", [128, 1], f32, kind="ExternalInput")
    ones_d = nc.dram_tensor("ones", [128, NK], bf16, kind="ExternalInput")
    out_d = nc.dram_tensor("out", [S, H], f32, kind="ExternalOutput")

    with tile.TileContext(nc) as tc:
        with (
            tc.tile_pool(name="big", bufs=1) as big,
            tc.tile_pool(name="small", bufs=1) as small,
            tc.tile_pool(name="pt", bufs=4) as ptp,
            tc.tile_pool(name="res", bufs=2) as resp,
            tc.tile_pool(name="psA", bufs=3, space="PSUM") as psA,
            tc.tile_pool(name="psO", bufs=2, space="PSUM") as psO,
        ):
            # ---- constants / weights ----
            ident = small.tile([128, 128], f32)
            make_identity(nc, ident)
            identb = small.tile([128, 128], bf16)
            nc.gpsimd.tensor_copy(out=identb, in_=ident)

            wvq = small.tile([128, DT, 128], f32r)
            nc.sync.dma_start(
                out=wvq,
                in_=wvq_d[:, :].rearrange("(t p) h -> p t h", p=128).bitcast(f32r),
            )
            wk0 = small.tile([128, DT, 128], f32r)
            nc.sync.dma_start(
                out=wk0,
                in_=wk0_d[:, :].rearrange("(t p) h -> p t h", p=128).bitcast(f32r),
            )
            bvq = small.tile([128, 1], f32)
            nc.sync.dma_start(out=bvq, in_=bvq_d[:, :])
            bk0 = small.tile([128, 1], f32)
            nc.sync.dma_start(out=bk0, in_=bk0_d[:, :])

            # ---- x^T, DMA'd in s-column chunks so early chunks arrive fast
            xT = big.tile([128, DT, S], f32r)
            for c in range(NQ):
                for dt in range(DT):
                    nc.sync.dma_start(
                        out=xT[:, dt, c * 512 : (c + 1) * 512],
                        in_=xT_d[
                            dt * 128 : (dt + 1) * 128, c * 512 : (c + 1) * 512
                        ].bitcast(f32r),
                    )

            # q/k: data rows 64-127, zeros rows 0-63 (k's zeros come from
            # the zero-padded weights; q's from one memset).
            qT = big.tile([128, S], bf16, tag="qT")
            kT = big.tile([128, S], bf16, tag="kT")
            vTlo = big.tile([H, S], f32, tag="vTlo")
            nc.gpsimd.memset(qT[:H, :], 0.0)

            v65 = big.tile([128, NK, H + 1], bf16)
            nc.sync.dma_start(out=v65[:, :, H : H + 1], in_=ones_d[:, :].unsqueeze(2))

            outqs = [None] * NQ
            pT0 = [None] * NKP  # first q-chunk P^T pair-tiles

            def emit_s_exp(kp, qc, pstore):
                """scores for k-tile pair kp against q-chunk qc + 1024-wide exp."""
                sc = psA.tile([128, 1024], f32, tag="a", name=f"sc{qc}_{kp}")
                for h2 in range(2):
                    kt = kp * 2 + h2
                    nc.tensor.matmul(
                        sc[:, h2 * 512 : (h2 + 1) * 512],
                        lhsT=kT[:, kt * 128 : (kt + 1) * 128],
                        rhs=qT[:, qc * 512 : (qc + 1) * 512],
                        start=True,
                        stop=True,
                    )
                pt = ptp.tile([128, 1024], bf16, tag="pT", name=f"pt{qc}_{kp}")
                nc.scalar.activation(out=pt, in_=sc, func=Exp, scale=SCALE)
                pstore[kp] = pt

            def emit_pv(kp, qc, pstore):
                for h2 in range(2):
                    kt = kp * 2 + h2
                    nc.tensor.matmul(
                        outqs[qc],
                        lhsT=v65[:, kt, :],
                        rhs=pstore[kp][:, h2 * 512 : (h2 + 1) * 512],
                        start=(kt == 0),
                        stop=(kt == NK - 1),
                    )

            def emit_epilogue(qc):
                oTq = resp.tile([H + 1, 512], bf16, tag="oT", name=f"oT{qc}")
                nc.vector.tensor_copy(out=oTq, in_=outqs[qc])
                tp4 = psA.tile([128, 4, H + 2], bf16, tag="a", name=f"tp4_{qc}")
                for st in range(4):
                    nc.tensor.transpose(
                        tp4[:, st, : H + 1],
                        oTq[:, st * 128 : (st + 1) * 128],
                        identb[: H + 1, : H + 1],
                    )
                rec = resp.tile([128, 4, 1], f32, tag="rec", name=f"rec{qc}")
                nc.vector.reciprocal(out=rec, in_=tp4[:, :, H : H + 1])
                res = resp.tile([128, 4, H], f32, tag="res", name=f"res{qc}")
                nc.vector.tensor_mul(
                    out=res, in0=tp4[:, :, :H], in1=rec.broadcast_to([128, 4, H])
                )
                nc.sync.dma_start(
                    out=out_d[qc * 512 : (qc + 1) * 512, :].rearrange(
                        "(st p) h -> p st h", p=128
                    ),
                    in_=res,
                )

            # ---- projection phase, chunk-interleaved, carrying qc=0 work
            outqs[0] = psO.tile([H + 1, 512], f32, tag="o", name="outq0")
            for c in range(NQ):
                # fused v+q projection for this s-chunk (M=128)
                psvq = psA.tile([128, 512], f32, tag="a", name=f"psvq{c}")
                for dt in range(DT):
                    nc.tensor.matmul(
                        psvq,
                        lhsT=wvq[:, dt, :],
                        rhs=xT[:, dt, c * 512 : (c + 1) * 512],
                        start=(dt == 0),
                        stop=(dt == DT - 1),
                    )
                nc.vector.tensor_scalar_add(
                    out=vTlo[:, c * 512 : (c + 1) * 512],
                    in0=psvq[:H, :],
                    scalar1=bvq[:H, :],
                )
                nc.vector.tensor_scalar_add(
                    out=qT[H:, c * 512 : (c + 1) * 512],
                    in0=psvq[H:, :],
                    scalar1=bvq[H:, :],
                )
                # k projection (M=128, zero-padded weights -> rows 0-63 zero)
                psk = psA.tile([128, 512], f32, tag="a", name=f"psk{c}")
                for dt in range(DT):
                    nc.tensor.matmul(
                        psk,
                        lhsT=wk0[:, dt, :],
                        rhs=xT[:, dt, c * 512 : (c + 1) * 512],
                        start=(dt == 0),
                        stop=(dt == DT - 1),
                    )
                nc.vector.tensor_scalar_add(
                    out=kT[:, c * 512 : (c + 1) * 512], in0=psk, scalar1=bk0
                )
                # V transposes for this chunk's 4 k-tiles
                for j in range(4):
                    kt = c * 4 + j
                    tp = psA.tile([128, H], f32, tag="a", name=f"vtr{kt}")
                    nc.tensor.transpose(
                        tp, vTlo[:, kt * 128 : (kt + 1) * 128], ident[:H, :H]
                    )
                    nc.vector.tensor_copy(out=v65[:, kt, :H], in_=tp)
                # first q-chunk scores/exp for this chunk's k-tiles;
                # PVs run one chunk behind so exp has a full window of lead
                for kp in (2 * c, 2 * c + 1):
                    emit_s_exp(kp, 0, pT0)
                if c >= 1:
                    emit_pv(2 * c - 2, 0, pT0)
                    emit_pv(2 * c - 1, 0, pT0)
            emit_pv(NKP - 2, 0, pT0)
            emit_pv(NKP - 1, 0, pT0)

            emit_epilogue(0)

            # ---- remaining q-chunks, PV pipelined two pairs behind ----
            for qc in range(1, NQ):
                pTq = [None] * NKP
                outqs[qc] = psO.tile(
                    [H + 1, 512], f32, tag="o", name=f"outq{qc}"
                )
                for kp in range(NKP + 2):
                    if kp >= 2:
                        emit_pv(kp - 2, qc, pTq)
                    if kp < NKP:
                        emit_s_exp(kp, qc, pTq)
                emit_epilogue(qc)

    nc.compile()
    return nc


def _get_nc():
    if "nc" not in _cache:
        _cache["nc"] = _build()
    return _cache["nc"]


def _prep_inputs(x, Wq, bq, Wk, bk, Wv, bv):
    import ml_dtypes

    x = np.asarray(x, dtype=np.float32)
    Wq = np.asarray(Wq, np.float32)
    Wk = np.asarray(Wk, np.float32)
    Wv = np.asarray(Wv, np.float32)
    z = np.zeros((D, H), np.float32)
    common = {
        "wvq": np.ascontiguousarray(np.concatenate([Wv, Wq], axis=1)),
        "wk0": np.ascontiguousarray(np.concatenate([z, Wk], axis=1)),
        "bvq": np.ascontiguousarray(
            np.concatenate(
                [np.asarray(bv, np.float32).ravel(),
                 np.asarray(bq, np.float32).ravel()]
            ).reshape(128, 1)
        ),
        "bk0": np.ascontiguousarray(
            np.concatenate(
                [np.zeros(H, np.float32), np.asarray(bk, np.float32).ravel()]
            ).reshape(128, 1)
        ),
        "ones": np.ones((128, NK), ml_dtypes.bfloat16),
    }
    return x, common


def kernel(x, Wq, bq, Wk, bk, Wv, bv, **_):
    from concourse.bass_utils import run_bass_kernel_spmd

    nc = _get_nc()
    x, common = _prep_inputs(x, Wq, bq, Wk, bk, Wv, bv)
    in_maps = [
        {"xT": np.ascontiguousarray(x[b].T), **common} for b in range(B)
    ]
    res = run_bass_kernel_spmd(nc, in_maps, core_ids=list(range(B)))
    return np.stack([res.results[b]["out"] for b in range(B)])


# revision 17
# speedup vs baseline: 1.3297x; 1.0657x over previous
"""Single-head attention on 8 Trainium2 NeuronCores, batch-sharded.

Per core (one batch element b), with x fed pre-transposed as xT [768, 2048]:

  v^T/q^T    via one fused [Wv|Wq] matmul (M=128, full PE array): psum
             rows 0-63 = v^T, rows 64-127 = q^T.
  k^T        via a host-padded [0|Wk] matmul (M=128): psum rows 0-63 = 0,
             rows 64-127 = k^T.
  q^T, k^T   stored bf16 in rows 64-127 of [128, 2048] tiles with zeros
             in rows 0-63: the K=128 contraction streams 2x faster than
             K=64 on TRN2, zeros pair with zeros, and every engine copy
             stays partition-aligned (no cross-partition moves).
  v^T        f32 rows 0-63, PE-transposed per k-tile into V [k, h] bf16
             with an appended ones column.
  scores^T   [k-tile=128, q] = K^T-tile x Q^T      (PE bf16 K=128 N=512)
  P^T        = exp(scores^T / 8)                   (ACT -> bf16, 1024-wide;
             no max-subtraction: |scores/8| <~ 2, exp is safe)
  out^T,den  = [V | 1] x P^T accumulated over k    (PE bf16; the ones
             column yields the softmax denominator row)
  out        = transpose(out^T) rows / denominator (PE + DVE), one DMA
             per 512-row block.

Scheduling: xT arrives in s-column chunks; the projection phase is
chunk-interleaved and also carries the first q-chunk's scores/exp (PVs
one chunk behind), so PE and ACT are busy while DMA streams. Remaining
q-chunks run with PV two k-pairs behind the scores, epilogues hiding in
the next chunk's window. PE never idles long enough for the HAM clock
gate to re-throttle. No inter-core communication.
"""

import numpy as np

B, S, D, H = 8, 2048, 768, 64
DT = D // 128  # 6 d-tiles
NQ = S // 512  # 4 q-chunks of 512
NK = S // 128  # 16 k-tiles of 128
NKP = NK // 2  # 8 k-tile pairs (one 1024-wide exp each)
SCALE = 1.0 / np.sqrt(H).item()

_cache = {}


def _build():
    import concourse.mybir as mybir
    import concourse.tile as tile
    from concourse import bacc
    from concourse.masks import make_identity

    f32 = mybir.dt.float32
    f32r = mybir.dt.float32r
    bf16 = mybir.dt.bfloat16
    Exp = mybir.ActivationFunctionType.Exp

    nc = bacc.Bacc(None)
    xT_d = nc.dram_tensor("xT", [D, S], f32, kind="ExternalInput")
    wvq_d = nc.dram_tensor("wvq", [D, 128], f32, kind="ExternalInput")
    wk0_d = nc.dram_tensor("wk0", [D, 128], f32, kind="ExternalInput")
    bvq_d = nc.dram_tensor("bvq", [128, 1], f32, kind="ExternalInput")
    bk0_d = nc.dram_tensor("bk0", [128, 1], f32, kind="ExternalInput")
    ones_d = nc.dram_tensor("ones", [128, NK], bf16, kind="ExternalInput")
    out_d = nc.dram_tensor("out", [S, H], f32, kind="ExternalOutput")

    with tile.TileContext(nc) as tc:
        with (
            tc.tile_pool(name="big", bufs=1) as big,
            tc.tile_pool(name="small", bufs=1) as small,
            tc.tile_pool(name="pt", bufs=4) as ptp,
            tc.tile_pool(name="res", bufs=2) as resp,
            tc.tile_pool(name="psA", bufs=3, space="PSUM") as psA,
            tc.tile_pool(name="psO", bufs=2, space="PSUM") as psO,
        ):
            # ---- constants / weights ----
            ident = small.tile([128, 128], f32)
            make_identity(nc, ident)
            identb = small.tile([128, 128], bf16)
            nc.gpsimd.tensor_copy(out=identb, in_=ident)

            wvq = small.tile([128, DT, 128], f32r)
            nc.sync.dma_start(
                out=wvq,
                in_=wvq_d[:, :].rearrange("(t p) h -> p t h", p=128).bitcast(f32r),
            )
            wk0 = small.tile([128, DT, 128], f32r)
            nc.sync.dma_start(
                out=wk0,
                in_=wk0_d[:, :].rearrange("(t p) h -> p t h", p=128).bitcast(f32r),
            )
            bvq = small.tile([128, 1], f32)
            nc.sync.dma_start(out=bvq, in_=bvq_d[:, :])
            bk0 = small.tile([128, 1], f32)
            nc.sync.dma_start(out=bk0, in_=bk0_d[:, :])

            # ---- x^T, DMA'd in s-column chunks so early chunks arrive fast
            xT = big.tile([128, DT, S], f32r)
            for c in range(NQ):
                for dt in range(DT):
                    nc.sync.dma_start(
                        out=xT[:, dt, c * 512 : (c + 1) * 512],
                        in_=xT_d[
                            dt * 128 : (dt + 1) * 128, c * 512 : (c + 1) * 512
                        ].bitcast(f32r),
                    )

            # q/k: data rows 64-127, zeros rows 0-63 (k's zeros come from
            # the zero-padded weights; q's from one memset).
            qT = big.tile([128, S], bf16, tag="qT")
            kT = big.tile([128, S], bf16, tag="kT")
            vTlo = big.tile([H, S], f32, tag="vTlo")
            nc.gpsimd.memset(qT[:H, :], 0.0)

            v65 = big.tile([128, NK, H + 1], bf16)
            nc.sync.dma_start(out=v65[:, :, H : H + 1], in_=ones_d[:, :].unsqueeze(2))

            outqs = [None] * NQ
            pT0 = [None] * NKP  # first q-chunk P^T pair-tiles

            def emit_s_exp(kp, qc, pstore):
                """scores for k-tile pair kp against q-chunk qc + 1024-wide exp."""
                sc = psA.tile([128, 1024], f32, tag="a", name=f"sc{qc}_{kp}")
                for h2 in range(2):
                    kt = kp * 2 + h2
                    nc.tensor.matmul(
                        sc[:, h2 * 512 : (h2 + 1) * 512],
                        lhsT=kT[:, kt * 128 : (kt + 1) * 128],
                        rhs=qT[:, qc * 512 : (qc + 1) * 512],
                        start=True,
                        stop=True,
                    )
                pt = ptp.tile([128, 1024], bf16, tag="pT", name=f"pt{qc}_{kp}")
                nc.scalar.activation(out=pt, in_=sc, func=Exp, scale=SCALE)
                pstore[kp] = pt

            def emit_pv(kp, qc, pstore):
                for h2 in range(2):
                    kt = kp * 2 + h2
                    nc.tensor.matmul(
                        outqs[qc],
                        lhsT=v65[:, kt, :],
                        rhs=pstore[kp][:, h2 * 512 : (h2 + 1) * 512],
                        start=(kt == 0),
                        stop=(kt == NK - 1),
                    )

            def emit_epilogue(qc):
                oTq = resp.tile([H + 1, 512], bf16, tag="oT", name=f"oT{qc}")
                nc.vector.tensor_copy(out=oTq, in_=outqs[qc])
                tp4 = psA.tile([128, 4, H + 2], bf16, tag="a", name=f"tp4_{qc}")
                for st in range(4):
                    nc.tensor.transpose(
                        tp4[:, st, : H + 1],
                        oTq[:, st * 128 : (st + 1) * 128],
                        identb[: H + 1, : H + 1],
                    )
                rec = resp.tile([128, 4, 1], f32, tag="rec", name=f"rec{qc}")
                nc.vector.reciprocal(out=rec, in_=tp4[:, :, H : H + 1])
                res = resp.tile([128, 4, H], f32, tag="res", name=f"res{qc}")
                nc.vector.tensor_mul(
                    out=res, in0=tp4[:, :, :H], in1=rec.broadcast_to([128, 4, H])
                )
                nc.sync.dma_start(
                    out=out_d[qc * 512 : (qc + 1) * 512, :].rearrange(
                        "(st p) h -> p st h", p=128
                    ),
                    in_=res,
                )

            # ---- projection phase, chunk-interleaved, carrying qc=0 work
            outqs[0] = psO.tile([H + 1, 512], f32, tag="o", name="outq0")
            for c in range(NQ):
                # fused v+q projection for this s-chunk (M=128)
                psvq = psA.tile([128, 512], f32, tag="a", name=f"psvq{c}")
                for dt in range(DT):
                    nc.tensor.matmul(
                        psvq,
                        lhsT=wvq[:, dt, :],
                        rhs=xT[:, dt, c * 512 : (c + 1) * 512],
                        start=(dt == 0),
                        stop=(dt == DT - 1),
                    )
                nc.vector.tensor_scalar_add(
                    out=vTlo[:, c * 512 : (c + 1) * 512],
                    in0=psvq[:H, :],
                    scalar1=bvq[:H, :],
                )
                nc.vector.tensor_scalar_add(
                    out=qT[H:, c * 512 : (c + 1) * 512],
                    in0=psvq[H:, :],
                    scalar1=bvq[H:, :],
                )
                # k projection (M=128, zero-padded weights -> rows 0-63 zero)
                psk = psA.tile([128, 512], f32, tag="a", name=f"psk{c}")
                for dt in range(DT):
                    nc.tensor.matmul(
                        psk,
                        lhsT=wk0[:, dt, :],
                        rhs=xT[:, dt, c * 512 : (c + 1) * 512],
                        start=(dt == 0),
                        stop=(dt == DT - 1),
                    )
                nc.vector.tensor_scalar_add(
                    out=kT[:, c * 512 : (c + 1) * 512], in0=psk, scalar1=bk0
                )
                # V transposes for this chunk's 4 k-tiles
                for j in range(4):
                    kt = c * 4 + j
                    tp = psA.tile([128, H], f32, tag="a", name=f"vtr{kt}")
                    nc.tensor.transpose(
                        tp, vTlo[:, kt * 128 : (kt + 1) * 128], ident[:H, :H]
                    )
                    nc.vector.tensor_copy(out=v65[:, kt, :H], in_=tp)
                # first q-chunk scores/exp for this chunk's k-tiles;
                # PVs run one chunk behind so exp has a full window of lead
                for kp in (2 * c, 2 * c + 1):
                    emit_s_exp(kp, 0, pT0)
                if c >= 1:
                    emit_pv(2 * c - 2, 0, pT0)
                    emit_pv(2 * c - 1, 0, pT0)

            # ---- remaining q-chunks as ONE continuous S/exp/PV stream ----
            # (PV trails the scores stream by 2 pairs globally, including
            # across q-chunk boundaries, so the PE never drains.)
            pts = {}
            for kp in range(NKP):
                pts[(0, kp)] = pT0[kp]
            s_tasks = [(qc, kp) for qc in range(1, NQ) for kp in range(NKP)]
            pv_tasks = [(0, NKP - 2), (0, NKP - 1)] + s_tasks
            for qc in range(1, NQ):
                outqs[qc] = psO.tile(
                    [H + 1, 512], f32, tag="o", name=f"outq{qc}"
                )
            for i in range(len(pv_tasks)):
                if i < len(s_tasks):
                    sqc, skp = s_tasks[i]
                    pst = [None] * NKP
                    emit_s_exp(skp, sqc, pst)
                    pts[(sqc, skp)] = pst[skp]
                pqc, pkp = pv_tasks[i]
                emit_pv(pkp, pqc, {pkp: pts[(pqc, pkp)]})
                if pkp == NKP - 1:
                    emit_epilogue(pqc)

    nc.compile()
    return nc


def _get_nc():
    if "nc" not in _cache:
        _cache["nc"] = _build()
    return _cache["nc"]


def _prep_inputs(x, Wq, bq, Wk, bk, Wv, bv):
    import ml_dtypes

    x = np.asarray(x, dtype=np.float32)
    Wq = np.asarray(Wq, np.float32)
    Wk = np.asarray(Wk, np.float32)
    Wv = np.asarray(Wv, np.float32)
    z = np.zeros((D, H), np.float32)
    common = {
        "wvq": np.ascontiguousarray(np.concatenate([Wv, Wq], axis=1)),
        "wk0": np.ascontiguousarray(np.concatenate([z, Wk], axis=1)),
        "bvq": np.ascontiguousarray(
            np.concatenate(
                [np.asarray(bv, np.float32).ravel(),
                 np.asarray(bq, np.float32).ravel()]
            ).reshape(128, 1)
        ),
        "bk0": np.ascontiguousarray(
            np.concatenate(
                [np.zeros(H, np.float32), np.asarray(bk, np.float32).ravel()]
            ).reshape(128, 1)
        ),
        "ones": np.ones((128, NK), ml_dtypes.bfloat16),
    }
    return x, common


def kernel(x, Wq, bq, Wk, bk, Wv, bv, **_):
    from concourse.bass_utils import run_bass_kernel_spmd

    nc = _get_nc()
    x, common = _prep_inputs(x, Wq, bq, Wk, bk, Wv, bv)
    in_maps = [
        {"xT": np.ascontiguousarray(x[b].T), **common} for b in range(B)
    ]
    res = run_bass_kernel_spmd(nc, in_maps, core_ids=list(range(B)))
    return np.stack([res.results[b]["out"] for b in range(B)])
